# revision 42
# baseline (speedup 1.0000x reference)
"""GraphSAGE (3x SAGEConv mean-agg + linear classifier + log_softmax) on 8
Trainium2 NeuronCores via Bass.

Self-contained: host-side packing + SPMD bass program + gather/unshard.

Sharding: nodes are dst-sharded 8 ways (core c owns nodes [c*NP, (c+1)*NP)).
Per layer, each core:
  - bulk-gathers its in-edges' source rows from a replicated fp16 DRAM
    feature table with InstDMAGatherAnt (1024 int16 indices per call, quad
    granularity: each 512B descriptor fetches 4 rows, the wanted one selected
    by a per-cell mask),
  - applies mask*(1/deg) on the Vector engine (broadcast multiply), then
    segment-reduces each degree-sorted rectangle with a contiguous
    pairwise fold tree,
  - PE-transposes the mean grid to feature-major, matmuls Wl/Wr with PSUM
    accumulation, applies bias+ReLU on the Scalar engine,
  - PE-transposes back to node-major (fp16) and AllGathers the shard into
    the next layer's fp16 table (compute/slot order).
The tiny 64-wide linears are replicated on every core.
"""
import os
import sys
import types

import numpy as np

sys.path.insert(0, "/opt/trn_rl_repo")

P = 8
F = 64
NCLS = 10
CW = 64          # gather-chunk width (8 full calls = 2 rounds of 4 queues)
GPC = 8          # grid columns per dma_gather call (8*128 = 1024 idx)
MMW = 512        # matmul moving-chunk width
QF = 256         # fp16 elems per quad cell (4 rows x 64)
MASK4D = int(os.environ.get("MASK4D", "1")) == 1

LAST_EXEC_NS = None


# ---------------------------------------------------------------- host packing

def _build_meta(edge_index, n_nodes):
    N = n_nodes
    NP = N // P
    PR = (NP + 127) // 128
    SLOTS = PR * 128
    assert NP < SLOTS, "need at least one dummy slot (NP not divisible by 128)"
    src = np.asarray(edge_index[0], dtype=np.int64)
    dst = np.asarray(edge_index[1], dtype=np.int64)
    deg = np.bincount(dst, minlength=N).astype(np.int64)

    orders = []
    deg_sorted = []
    for c in range(P):
        dl = deg[c * NP:(c + 1) * NP]
        o = np.argsort(-dl, kind="stable")
        orders.append(o)
        deg_sorted.append(dl[o])
    w_prow = np.zeros(PR, dtype=np.int64)
    for i in range(PR):
        w_prow[i] = max(1, max(int(ds[i * 128]) for ds in deg_sorted))

    # Bin-pack prow-blocks into chunks of width exactly CW (= 4 full
    # dma_gather calls, one per SWDGE queue -> perfectly balanced queues).
    # Blocks wider than CW are capped: the first CW neighbor cells stay in
    # the main (full) item, the overflow cells go to a small pseudo-block
    # placed later that fold-ACCUMULATES into the same mean_t rows.
    # Blocks are consumed out of degree order; slots renumbered to match.
    entries = []                     # (origblock, width, kind)
    pend_ov = []                     # ov pieces waiting for next chunk
    for b in range(PR):
        w = int(w_prow[b])
        if w > CW:
            entries.append((b, CW, "main"))
        else:
            entries.append((b, w, "main"))
    entries.sort(key=lambda e: -e[1])
    remaining = list(entries)
    packed = []                      # list of entry-lists
    while remaining or pend_ov:
        rem = CW
        members = []
        # overflow pieces first: their mains are in earlier chunks, and
        # finalization of their prows (and AllGather pieces) waits on them
        while pend_ov and pend_ov[0][1] <= rem:
            members.append(pend_ov.pop(0))
            rem -= members[-1][1]
        last_w = None
        while rem > 0 and remaining:
            # prefer a block matching the previous width (bigger fold
            # m-groups), else the largest that fits
            k = None
            if last_w is not None and last_w <= rem:
                k = next((j for j, e in enumerate(remaining)
                          if e[1] == last_w), None)
            if k is None:
                k = next((j for j, e in enumerate(remaining)
                          if e[1] <= rem), None)
            if k is None:
                break
            e = remaining.pop(k)
            members.append(e)
            rem -= e[1]
            last_w = e[1]
            b = e[0]
            if e[2] == "main" and int(w_prow[b]) > CW:
                pend_ov.append((b, int(w_prow[b]) - CW, "ov"))
        packed.append(members)

    # new prow numbering = placement order of MAIN entries
    newprow = {}
    blockseq = []
    for members in packed:
        for b, w, kind in members:
            if kind == "main":
                newprow[b] = len(blockseq)
                blockseq.append(b)
    assert len(blockseq) == PR
    w_main = np.array([min(int(w_prow[b]), CW) for b in blockseq],
                      dtype=np.int64)

    c0_prow = np.zeros(PR, dtype=np.int64)     # main cells, by NEW prow
    c0_ov = {}                                  # overflow cells, by NEW prow
    complete_chunk = np.zeros(PR, dtype=np.int64)
    chunks = []
    col = 0
    for ci, members in enumerate(packed):
        items = []
        coff = 0
        j = 0
        while j < len(members):
            b, w, kind = members[j]
            if kind == "main":
                i0 = newprow[b]
                m = 1
                while (j + m < len(members)
                       and members[j + m][2] == "main"
                       and members[j + m][1] == w
                       and newprow[members[j + m][0]] == i0 + m):
                    m += 1
                for k in range(m):
                    c0_prow[i0 + k] = col + coff + k * w
                    complete_chunk[i0 + k] = max(complete_chunk[i0 + k], ci)
                items.append({"kind": "full", "coff": coff, "i0": i0,
                              "m": m, "w": w})
                coff += m * w
                j += m
            else:
                i0 = newprow[b]
                c0_ov[i0] = col + coff
                complete_chunk[i0] = max(complete_chunk[i0], ci)
                items.append({"kind": "part", "coff": coff, "i0": i0,
                              "m": 1, "w": w, "first": False})
                coff += w
                j += 1
        chunks.append({"c0": col, "width": CW, "items": items})
        col += CW
    C = col

    fin = 0
    for ci, ch in enumerate(chunks):
        fin1 = fin
        while fin1 < PR and complete_chunk[fin1] <= ci:
            fin1 += 1
        ch["fin0"], ch["fin1"] = fin, fin1
        fin = fin1
    assert fin == PR

    col_prow = np.full(C, -1, dtype=np.int64)
    for i in range(PR):
        col_prow[c0_prow[i]:c0_prow[i] + int(w_main[i])] = i
    for i, c0o in c0_ov.items():
        wov = int(w_prow[blockseq[i]]) - CW
        col_prow[c0o:c0o + wov] = i
    w_full = np.array([int(w_prow[b]) for b in blockseq], dtype=np.int64)

    # slot (new prow i, p) <- degree-rank blockseq[i]*128 + p
    rank_of_slot = np.full(SLOTS, -1, dtype=np.int64)
    for i, b in enumerate(blockseq):
        r0 = b * 128
        n = min(128, NP - r0)
        if n > 0:
            rank_of_slot[i * 128:i * 128 + n] = np.arange(r0, r0 + n)

    grids = []
    slot_nodes = []
    slot_of = np.full(N, -1, dtype=np.int64)
    for c in range(P):
        o = orders[c]
        ros = np.maximum(rank_of_slot, 0)
        slot_node = np.where(rank_of_slot >= 0, o[ros] + c * NP, -1)
        slot_nodes.append(slot_node)
        slot_of[slot_node[slot_node >= 0]] = np.nonzero(slot_node >= 0)[0]

        grid = np.full((128, C), -1, dtype=np.int64)
        m = (dst >= c * NP) & (dst < (c + 1) * NP)
        es, ed = src[m], dst[m] - c * NP
        eo = np.argsort(ed, kind="stable")
        es, ed = es[eo], ed[eo]
        estart = np.zeros(NP + 1, dtype=np.int64)
        np.cumsum(np.bincount(ed, minlength=NP), out=estart[1:])
        r_e = slot_of[ed + c * NP]
        k_e = np.arange(es.shape[0]) - estart[ed]
        pp = r_e % 128
        pr_e = r_e // 128
        ovb = np.array([c0_ov.get(i, 0) for i in range(PR)],
                       dtype=np.int64)[pr_e]
        wm = w_main[pr_e]
        cols = np.where(k_e < wm, c0_prow[pr_e] + k_e, ovb + (k_e - wm))
        grid[pp, cols] = es
        grids.append(grid)

    dummy = np.nonzero(rank_of_slot < 0)[0]
    dummy_rng = (int(dummy.min()), int(dummy.max()) + 1) if dummy.size else (0, 0)
    assert dummy.size == dummy_rng[1] - dummy_rng[0], "dummy slots contiguous"

    return {
        "N": N, "NP": NP, "PR": PR, "SLOTS": SLOTS, "C": C,
        "chunks": chunks, "grids": grids, "slot_nodes": slot_nodes,
        "slot_of": slot_of, "deg": deg, "col_prow": col_prow,
        "dummy_rng": dummy_rng,
    }


def _wrap_idx(q, chunks):
    """q: [128, C] int idx grid -> [128, C*8] int16 call-wrapped layout.

    Gather calls cover GPC grid columns starting at each chunk's c0; per
    call, flat k' = jl*128+p -> staged [k'%16, (c0+s0)*8 + k'//16],
    replicated across the 8 partition groups.
    """
    C = q.shape[1]
    out = np.zeros((16, C * 8), dtype=np.int16)
    for ch in chunks:
        for s0 in range(0, ch["width"], GPC):
            cw = min(GPC, ch["width"] - s0)
            g0 = ch["c0"] + s0
            flat = q[:, g0:g0 + cw].T.reshape(-1)      # k' = jl*128 + p
            blk = flat.reshape(cw * 8, 16).T           # [16, cw*8]
            out[:, g0 * 8:g0 * 8 + cw * 8] = blk
    return np.tile(out, (8, 1))                        # [128, C*8]


def _ag_pieces(PR):
    """AllGather piece boundaries in prows (staggered, small final piece so
    the next layer's gathers aren't stuck behind one big late collective).

    Returns (piece_lo_slots, piece_size_slots, piece_base_gids) as arrays.
    """
    cuts = sorted(set([0, min(49, PR), min(73, PR), min(87, PR),
                   min(95, PR), PR]))
    lo = np.array([c * 128 for c in cuts[:-1]], dtype=np.int64)
    sz = np.array([(cuts[i + 1] - cuts[i]) * 128 for i in range(len(cuts) - 1)],
                  dtype=np.int64)
    base = np.concatenate([[0], np.cumsum(sz * P)[:-1]])
    return lo, sz, base


def _build_core_inputs(meta, x):
    N, NP, PR, SLOTS, C = (meta[k] for k in ("N", "NP", "PR", "SLOTS", "C"))
    T2 = P * SLOTS
    inv = 1.0 / np.maximum(meta["deg"], 1).astype(np.float32)
    p_lo, p_sz, p_base = _ag_pieces(PR)

    # layer-1 feature table in AllGather gid order (same layout the kernel's
    # AllGather produces for layers 2/3, so one idx/mask set serves all)
    assert T2 % 4 == 0
    htbl = np.zeros((T2, F), np.float16)
    for c in range(P):
        sn = meta["slot_nodes"][c]
        for k in range(len(p_lo)):
            lo, sz = int(p_lo[k]), int(p_sz[k])
            slots = np.arange(lo, lo + sz)
            gids = int(p_base[k]) + c * sz + (slots - lo)
            real = sn[slots] >= 0
            htbl[gids[real]] = x[sn[slots][real]].astype(np.float16)

    per_core = []
    for c in range(P):
        grid = meta["grids"][c]                             # [128, C] node or -1
        valid = grid >= 0
        node = np.where(valid, grid, 0)

        owner = node // NP
        slot = meta["slot_of"][node]
        k = np.searchsorted(p_lo, slot, side="right") - 1
        gid = p_base[k] + owner * p_sz[k] + (slot - p_lo[k])
        q2 = (gid // 4).astype(np.int16)
        m2 = (gid % 4).astype(np.int64)

        # inv-degree of each cell's dst (by its (p, prow) position)
        slot_node = meta["slot_nodes"][c]
        prow = np.maximum(meta["col_prow"], 0)               # [C] prow of col
        dslot = prow[None, :] * 128 + np.arange(128)[:, None]  # [128, C] slot
        dn = slot_node[dslot]
        cinv = np.where(dn >= 0, inv[np.maximum(dn, 0)], 0.0).astype(np.float32)
        cinv = cinv * valid

        msk = np.zeros((128, C, 4), np.float16)
        pp, cc2 = np.nonzero(valid)
        msk[pp, cc2, m2[pp, cc2]] = cinv[pp, cc2]
        # duplicated feature-pair layout [128, C*4, 2] for 2x-mode DVE mul
        mskd = np.repeat(msk.reshape(128, C * 4, 1), 2, axis=2)

        rr = np.arange(SLOTS)
        real = slot_node >= 0
        xfm = np.zeros((F, SLOTS), np.float32)
        xfm[:, rr[real]] = x[slot_node[real]].T
        per_core.append({
            "idx": _wrap_idx(q2, meta["chunks"]),
            "msk": np.ascontiguousarray(mskd),
            "xfm": xfm, "htbl": htbl,
        })
    return per_core


# ---------------------------------------------------------------- bass builder

def _dma_gather_raw(gp, out_ap, in_ap, idxs_ap, num_idxs, elem_size,
                    elem_step=None, queue_num=0, num_idxs_reg=None):
    """bass dma_gather minus the elem%256B transpose-only restriction."""
    import concourse.mybir as mybir
    from concourse import ap_utils
    from concourse._compat import exact_div

    if num_idxs_reg is None:
        num_idxs_reg = num_idxs

    assert idxs_ap.dtype == mybir.dt.int16
    assert in_ap.dtype == out_ap.dtype
    if elem_step is None:
        assert ap_utils.ap_is_contiguous(in_ap.ap[1:])
        elem_step = elem_size
    assert ap_utils.ap_is_contiguous(out_ap.ap[1:])
    assert ap_utils.ap_is_contiguous(idxs_ap.ap[1:])
    assert in_ap.ap[-1][1] == out_ap.ap[-1][1] == elem_size
    assert out_ap.ap[0][1] * out_ap.ap[1][1] == num_idxs
    assert in_ap.ap[0][0] == elem_step
    stride_bytes_256 = exact_div(elem_step * mybir.dt.size(in_ap.dtype), 256)
    assert stride_bytes_256 < 256

    _in_ap = gp.lower_ap_dma(in_ap, for_custom_bir_dma=True)
    _idxs_ap = gp.lower_ap(idxs_ap)
    _out_ap = gp.lower_ap(out_ap)
    return gp.add_instruction(
        mybir.InstDMAGatherAnt(
            name=gp.bass.get_next_instruction_name(),
            ins=[*_in_ap, _idxs_ap,
                 gp.lower_val_access(gp.to_reg(num_idxs_reg))],
            outs=[_out_ap],
            transpose=False,
            num_idxs=num_idxs,
            elem_size=elem_size,
            stride_bytes_256=stride_bytes_256,
            gen_mode=0,
            single_packet=True,
            queue_num=queue_num,
            sbuf_tokens_per_rank=0,
            sbuf_free_dim_per_rank=0,
            sbuf_free_dim_pad_per_rank=0,
            sbuf_byte_offset=0,
        )
    )


def _build_bass(meta, n_cores=P):
    from concourse import bacc, tile, mybir

    N, NP, PR, SLOTS, C = (meta[k] for k in ("N", "NP", "PR", "SLOTS", "C"))
    T2 = P * SLOTS
    f32 = mybir.dt.float32
    f16 = mybir.dt.float16
    i16 = mybir.dt.int16
    AF = mybir.ActivationFunctionType
    OP = mybir.AluOpType
    AX = mybir.AxisListType

    nc = bacc.Bacc("TRN2", target_bir_lowering=False, debug=False,
                   num_devices=n_cores, num_swdge_queues=4,
                   dynamic_dma_scratch_size=4096)
    htbl = nc.dram_tensor("htbl", [T2, F], f16, kind="ExternalInput")
    idx_d = nc.dram_tensor("idx", [128, C * 8], i16, kind="ExternalInput")
    msk_d = nc.dram_tensor("msk", [128, C * 4, 2], f16, kind="ExternalInput")
    xfm_d = nc.dram_tensor("xfm", [F, SLOTS], f32, kind="ExternalInput")
    ident_d = nc.dram_tensor("ident", [128, 128], f32, kind="ExternalInput")
    ident16_d = nc.dram_tensor("ident16", [128, 128], f16, kind="ExternalInput")
    wts = {}
    for i in (1, 2, 3):
        wts[f"Wl{i}"] = nc.dram_tensor(f"Wl{i}", [F, F], f32, kind="ExternalInput")
        wts[f"Wr{i}"] = nc.dram_tensor(f"Wr{i}", [F, F], f32, kind="ExternalInput")
        wts[f"bl{i}"] = nc.dram_tensor(f"bl{i}", [F, 1], f32, kind="ExternalInput")
    wts["Wc"] = nc.dram_tensor("Wc", [F, NCLS], f32, kind="ExternalInput")
    wts["bc"] = nc.dram_tensor("bc", [128, NCLS], f32, kind="ExternalInput")
    out_d = nc.dram_tensor("out", [SLOTS, NCLS], f32, kind="ExternalOutput")

    maxpr = max(ch["fin1"] - ch["fin0"] for ch in meta["chunks"])
    pc_lo, pc_sz, pc_base = _ag_pieces(PR)

    with tile.TileContext(nc) as tc:
        from contextlib import ExitStack
        with ExitStack() as es:
            dram = es.enter_context(tc.tile_pool(name="dram", bufs=1, space="DRAM"))
            const = es.enter_context(tc.tile_pool(name="const", bufs=1))
            gbuf = es.enter_context(tc.tile_pool(name="gbuf", bufs=3))
            mpool = es.enter_context(tc.tile_pool(name="mpool", bufs=3))
            psT = es.enter_context(tc.tile_pool(name="psT", bufs=2, space="PSUM"))
            psM = es.enter_context(tc.tile_pool(name="psM", bufs=2, space="PSUM"))
            psN = es.enter_context(tc.tile_pool(name="psN", bufs=2, space="PSUM"))

            agin_t = dram.tile([SLOTS, F], f16, tag="agin", name="agin")
            agout_a = dram.tile([T2, F], f16, tag="agout_a", name="agout_a")
            agout_b = dram.tile([T2, F], f16, tag="agout_b", name="agout_b")
            hfm_a = dram.tile([F, SLOTS], f32, tag="hfm_a", name="hfm_a")
            hfm_b = dram.tile([F, SLOTS], f32, tag="hfm_b", name="hfm_b")
            hfm_dram = [hfm_a, hfm_b]

            idx_t = const.tile([128, C * 8], i16, tag="idx", name="idx_t")
            nc.sync.dma_start(idx_t[:], idx_d[:])
            msk_t = const.tile([128, C * 4, 2], f16, tag="msk", name="msk_t")
            nc.sync.dma_start(msk_t[:], msk_d[:])
            ident_t = const.tile([128, 128], f32, tag="ident", name="ident_t")
            nc.sync.dma_start(ident_t[:], ident_d[:])
            ident16_t = const.tile([128, 128], f16, tag="ident16",
                                   name="ident16_t")
            nc.sync.dma_start(ident16_t[:], ident16_d[:])
            w_t = {}
            for k, dten in wts.items():
                wtile = const.tile(list(dten.shape), f32, tag=k, name=f"w_{k}")
                w_t[k] = wtile
                nc.sync.dma_start(wtile[:], dten[:])

            mean_t = const.tile([128, PR, F], f16, tag="mean", name="mean_t")
            ngrid_t = const.tile([128, PR, F], f16, tag="ngrid", name="ngrid_t")
            ogrid_t = const.tile([128, PR, NCLS], f32, tag="ogrid",
                                 name="ogrid_t")
            ogrid2_t = const.tile([128, PR, NCLS], f32, tag="ogrid2",
                                  name="ogrid2_t")

            for L in range(3):
                if L == 0:
                    table_q = htbl[:].rearrange("(q g) f -> q (g f)", g=4)
                elif L == 1:
                    table_q = agout_a[:].rearrange("(q g) f -> q (g f)", g=4)
                else:
                    table_q = agout_b[:].rearrange("(q g) f -> q (g f)", g=4)
                agout_t = agout_a if L == 0 else agout_b
                hin = xfm_d if L == 0 else hfm_dram[(L + 1) % 2]
                hout = hfm_dram[L % 2]
                Wl, Wr, bl = w_t[f"Wl{L+1}"], w_t[f"Wr{L+1}"], w_t[f"bl{L+1}"]

                def send_piece(k):
                    lo = int(pc_lo[k])
                    szs = int(pc_sz[k])
                    base = int(pc_base[k])
                    pr_lo, pr_n = lo // 128, szs // 128
                    nc.sync.dma_start(
                        agin_t[lo:lo + szs, :].rearrange(
                            "(i p) f -> p i f", p=128),
                        ngrid_t[:, pr_lo:pr_lo + pr_n, :])
                    nc.gpsimd.collective_compute(
                        "AllGather", OP.bypass,
                        ins=[agin_t[lo:lo + szs, :].opt()],
                        outs=[agout_t[base:base + n_cores * szs, :].opt()],
                        replica_groups=[list(range(n_cores))])

                pieces_sent = 0
                qn = 0
                for ch in meta["chunks"]:
                    W = ch["width"]
                    c0 = ch["c0"]
                    buf = gbuf.tile([128, CW, QF], f16, tag="chunk", name="buf")
                    for s0 in range(0, W, GPC):
                        cw = min(GPC, W - s0)
                        _dma_gather_raw(
                            nc.gpsimd, buf[:, s0:s0 + cw, :], table_q,
                            idx_t[:, (c0 + s0) * 8:(c0 + s0 + cw) * 8],
                            cw * 128, QF, queue_num=qn)
                        qn = (qn + 1) % 4
                    # mask * inv-degree per chunk (zeroes junk quad rows +
                    # pad)
                    if MASK4D:
                        # duplicated-pair mask keeps every operand's innermost
                        # dim a packed 2-elem fp16 run -> DVE 2x mode
                        v = buf[:, 0:W, :].rearrange(
                            "p w (g f2 two) -> p (w g) f2 two", g=4, two=2)
                        mv = msk_t[:, c0 * 4:(c0 + W) * 4, :].rearrange(
                            "p m (one two) -> p m one two", one=1).broadcast_to(
                            [128, W * 4, F // 2, 2])
                        nc.vector.tensor_mul(v, v, mv)
                    else:
                        v = buf[:, 0:W, :].rearrange(
                            "p w (g f) -> p (w g) f", g=4)
                        nc.vector.tensor_mul(
                            v, v, msk_t[:, c0 * 4:(c0 + W) * 4, 0:1]
                            .broadcast_to([128, W * 4, F]))
                    # segment-reduce each item by pairwise folding
                    for r in ch["items"]:
                        m = r.get("m", 1)
                        w, i0 = r["w"], r["i0"]
                        D = buf[:, r["coff"]:r["coff"] + m * w, :].rearrange(
                            "p (m w) (g f) -> p m (w g) f", m=m, w=w, g=4)
                        X = 4 * w
                        while X > 2:
                            h = (X + 1) // 2
                            lo = X - h
                            nc.vector.tensor_add(D[:, :, 0:lo, :],
                                                 D[:, :, 0:lo, :],
                                                 D[:, :, h:X, :])
                            X = h
                        if r["kind"] == "full" or r["first"]:
                            nc.vector.tensor_add(mean_t[:, i0:i0 + m, :],
                                                 D[:, :, 0, :], D[:, :, 1, :])
                        else:
                            # tail of a prow split across chunks: accumulate
                            nc.vector.tensor_add(D[:, :, 0, :], D[:, :, 0, :],
                                                 D[:, :, 1, :])
                            nc.vector.tensor_add(mean_t[:, i0:i0 + m, :],
                                                 mean_t[:, i0:i0 + m, :],
                                                 D[:, :, 0, :])
                    # finalized prows: transpose, matmul now
                    i0c = ch["fin0"]
                    i1c = ch["fin1"]
                    npr = i1c - i0c
                    if npr == 0:
                        continue
                    s0 = i0c * 128
                    wd = npr * 128
                    mfm = mpool.tile([F, maxpr * 128], f32, tag="mfm", name="mfm")
                    for k in range(npr):
                        i = i0c + k
                        ps = psT.tile([F, 128], f16, tag="psT", name="psTt")
                        nc.tensor.transpose(ps[:], mean_t[:, i, :],
                                            ident16_t[:])
                        nc.scalar.activation(mfm[:, k * 128:(k + 1) * 128],
                                             ps[:], AF.Copy)
                    hin_sb = mpool.tile([F, maxpr * 128], f32, tag="hin",
                                        name="hin_sb")
                    nc.sync.dma_start(hin_sb[:, :wd], hin[:, s0:s0 + wd])
                    for q0 in range(0, wd, MMW):
                        qw = min(MMW, wd - q0)
                        ps = psM.tile([F, MMW], f32, tag="psM", name="psMt")
                        nc.tensor.matmul(ps[:, :qw], Wl[:],
                                         mfm[:, q0:q0 + qw],
                                         start=True, stop=False)
                        nc.tensor.matmul(ps[:, :qw], Wr[:],
                                         hin_sb[:, q0:q0 + qw],
                                         start=False, stop=True)
                        nc.scalar.activation(mfm[:, q0:q0 + qw], ps[:, :qw],
                                             AF.Relu, bias=bl[:])
                    hout_sb = mfm   # relu result written back into mfm tile
                    if L == 2:
                        for k in range(npr):
                            i = i0c + k
                            psc = psN.tile([128, NCLS], f32, tag="psN",
                                           name="psct")
                            nc.tensor.matmul(
                                psc[:], hout_sb[:, k * 128:(k + 1) * 128],
                                w_t["Wc"][:], start=True, stop=True)
                            nc.vector.tensor_add(ogrid_t[:, i, :], psc[:],
                                                 w_t["bc"][:])
                    else:
                        dlo = max(s0, meta["dummy_rng"][0])
                        dhi = min(s0 + wd, meta["dummy_rng"][1])
                        if dlo < dhi:
                            nc.vector.memset(
                                hout_sb[:, dlo - s0:dhi - s0], 0.0)
                        nc.sync.dma_start(hout[:, s0:s0 + wd], hout_sb[:, :wd])
                        for k in range(npr):
                            i = i0c + k
                            psn = psN.tile([128, F], f32, tag="psN", name="psnt")
                            nc.tensor.transpose(
                                psn[:], hout_sb[:, k * 128:(k + 1) * 128],
                                ident_t[:F, :F])
                            nc.scalar.activation(ngrid_t[:, i, :], psn[:],
                                                 AF.Copy)
                        while (pieces_sent < len(pc_lo) - 1 and
                               i1c * 128 >= pc_lo[pieces_sent] + pc_sz[pieces_sent]):
                            send_piece(pieces_sent)
                            pieces_sent += 1

                if L < 2:
                    while pieces_sent < len(pc_lo):
                        send_piece(pieces_sent)
                        pieces_sent += 1

            mx = const.tile([128, PR, 1], f32, tag="mx", name="mx")
            nc.vector.tensor_reduce(mx[:], ogrid_t[:], AX.X, OP.max)
            nc.vector.tensor_sub(ogrid2_t[:], ogrid_t[:],
                                 mx[:].broadcast_to([128, PR, NCLS]))
            eg = const.tile([128, PR, NCLS], f32, tag="eg", name="eg")
            nc.scalar.activation(eg[:], ogrid2_t[:], AF.Exp)
            sm = const.tile([128, PR, 1], f32, tag="sm", name="sm")
            nc.vector.tensor_reduce(sm[:], eg[:], AX.X, OP.add)
            lsm = const.tile([128, PR, 1], f32, tag="lsm", name="lsm")
            nc.scalar.activation(lsm[:], sm[:], AF.Ln)
            nc.vector.tensor_sub(ogrid_t[:], ogrid2_t[:],
                                 lsm[:].broadcast_to([128, PR, NCLS]))
            nc.sync.dma_start(out_d[:].rearrange("(i p) c -> p i c", p=128),
                              ogrid_t[:])
    nc.compile()
    return nc


def _install_ntff_hook():
    mod = types.ModuleType("antenv.axon_hooks")
    def s(h):
        mod._hook = h
    def g():
        return getattr(mod, "_hook", None)
    mod.set_axon_ntff_profile_hook = s
    mod.get_axon_ntff_profile_hook = g
    sys.modules["antenv.axon_hooks"] = mod
    import antenv
    antenv.axon_hooks = mod
    from trn_agent_boot.trn_boot import _ntff_profile_via_ctypes
    s(_ntff_profile_via_ctypes("/opt/axon/libaxon_pjrt.so"))


def kernel(**inputs):
    global LAST_EXEC_NS
    from concourse import bass_utils
    from concourse.bass_interp import get_hw_module

    x = np.asarray(inputs["x"], np.float32)
    edge_index = np.asarray(inputs["edge_index"], np.int64)
    N = x.shape[0]

    meta = _build_meta(edge_index, N)
    per_core = _build_core_inputs(meta, x)
    nc = _build_bass(meta, n_cores=P)
    nc.m = get_hw_module(nc.m)

    ident = np.eye(128, dtype=np.float32)
    ins = []
    for c in range(P):
        pc = per_core[c]
        m = {"htbl": pc["htbl"], "idx": pc["idx"], "msk": pc["msk"],
             "xfm": pc["xfm"], "ident": ident,
             "ident16": ident.astype(np.float16)}
        for i in (1, 2, 3):
            m[f"Wl{i}"] = np.asarray(inputs[f"Wl{i}"], np.float32)
            m[f"Wr{i}"] = np.asarray(inputs[f"Wr{i}"], np.float32)
            m[f"bl{i}"] = np.asarray(inputs[f"bl{i}"],
                                     np.float32).reshape(F, 1)
        m["Wc"] = np.asarray(inputs["Wc"], np.float32)
        m["bc"] = np.tile(np.asarray(inputs["bc"], np.float32).reshape(1, NCLS),
                          (128, 1))
        ins.append(m)

    trace = os.environ.get("KERNEL_TRACE", "0") == "1"
    if trace:
        try:
            _install_ntff_hook()
        except Exception:
            trace = False
    res = bass_utils.run_bass_kernel_spmd(
        nc, ins, core_ids=list(range(P)), trace=trace)
    LAST_EXEC_NS = res.exec_time_ns

    full = np.zeros((N, NCLS), np.float32)
    for c in range(P):
        sn = meta["slot_nodes"][c]
        real = sn >= 0
        full[sn[real]] = res.results[c]["out"][real]
    return full



# revision 43
# speedup vs baseline: 1.3539x; 1.3539x over previous
"""GraphSAGE (3x SAGEConv mean-agg + linear classifier + log_softmax) on 8
Trainium2 NeuronCores via Bass.

Self-contained: host-side packing + SPMD bass program + gather/unshard.

Sharding: nodes are dst-sharded 8 ways (core c owns nodes [c*NP, (c+1)*NP)).
Per layer, each core:
  - bulk-gathers its in-edges' source rows from a replicated fp16 DRAM
    feature table with InstDMAGatherAnt (1024 int16 indices per call, quad
    granularity: each 512B descriptor fetches 4 rows, the wanted one selected
    by a per-cell mask),
  - applies mask*(1/deg) on the Vector engine (broadcast multiply), then
    segment-reduces each degree-sorted rectangle with a contiguous
    pairwise fold tree,
  - PE-transposes the mean grid to feature-major, matmuls Wl/Wr with PSUM
    accumulation, applies bias+ReLU on the Scalar engine,
  - PE-transposes back to node-major (fp16) and AllGathers the shard into
    the next layer's fp16 table (compute/slot order).
The tiny 64-wide linears are replicated on every core.
"""
import os
import sys
import types

import numpy as np

sys.path.insert(0, "/opt/trn_rl_repo")

P = 8
F = 64
NCLS = 10
CW = 36          # gather-chunk width (grid columns per SBUF chunk buffer)
GPC = 8          # grid columns per dma_gather call (8*128 = 1024 idx)
MMW = 512        # matmul moving-chunk width
QF = 256         # fp16 elems per quad cell (4 rows x 64)
MASK4D = int(os.environ.get("MASK4D", "1")) == 1

LAST_EXEC_NS = None


# ---------------------------------------------------------------- host packing

def _build_meta(edge_index, n_nodes):
    N = n_nodes
    NP = N // P
    PR = (NP + 127) // 128
    SLOTS = PR * 128
    assert NP < SLOTS, "need at least one dummy slot (NP not divisible by 128)"
    src = np.asarray(edge_index[0], dtype=np.int64)
    dst = np.asarray(edge_index[1], dtype=np.int64)
    deg = np.bincount(dst, minlength=N).astype(np.int64)

    orders = []
    deg_sorted = []
    for c in range(P):
        dl = deg[c * NP:(c + 1) * NP]
        o = np.argsort(-dl, kind="stable")
        orders.append(o)
        deg_sorted.append(dl[o])
    w_prow = np.zeros(PR, dtype=np.int64)
    for i in range(PR):
        w_prow[i] = max(1, max(int(ds[i * 128]) for ds in deg_sorted))

    # pack prow-rects into chunks of width exactly CW (multiple of GPC) so
    # every dma_gather call is full-width; equal-w prow runs are split freely
    chunks = []
    col = 0
    cur = {"c0": 0, "width": 0, "rects": []}
    i = 0
    while i < PR:
        w = int(w_prow[i])
        assert w <= CW, f"prow width {w} exceeds chunk width {CW}"
        if cur["width"] + w > CW:
            chunks.append(cur)
            cur = {"c0": col, "width": 0, "rects": []}
        m = 1
        while (i + m < PR and int(w_prow[i + m]) == w
               and cur["width"] + (m + 1) * w <= CW):
            m += 1
        cur["rects"].append({"coff": cur["width"], "i0": i, "m": m, "w": w})
        cur["width"] += m * w
        col += m * w
        i += m
    if cur["width"] > 0:
        chunks.append(cur)
    C = col

    cc = 0
    for ch in chunks:
        ch["c0"] = cc
        cc += ch["width"]
    c0_prow = np.zeros(PR, dtype=np.int64)
    col_prow = np.zeros(C, dtype=np.int64)
    for ch in chunks:
        for r in ch["rects"]:
            for k in range(r["m"]):
                c0 = ch["c0"] + r["coff"] + k * r["w"]
                c0_prow[r["i0"] + k] = c0
                col_prow[c0:c0 + r["w"]] = r["i0"] + k

    grids = []
    slot_nodes = []
    slot_of = np.full(N, -1, dtype=np.int64)
    for c in range(P):
        o = orders[c]
        slot_node = np.full(SLOTS, -1, dtype=np.int64)
        slot_node[:NP] = o + c * NP
        slot_nodes.append(slot_node)
        slot_of[o + c * NP] = np.arange(NP)

        grid = np.full((128, C), -1, dtype=np.int64)
        m = (dst >= c * NP) & (dst < (c + 1) * NP)
        es, ed = src[m], dst[m] - c * NP
        eo = np.argsort(ed, kind="stable")
        es, ed = es[eo], ed[eo]
        estart = np.zeros(NP + 1, dtype=np.int64)
        np.cumsum(np.bincount(ed, minlength=NP), out=estart[1:])
        r_e = slot_of[ed + c * NP]
        k_e = np.arange(es.shape[0]) - estart[ed]
        pp = r_e % 128
        cols = c0_prow[r_e // 128] + k_e
        grid[pp, cols] = es
        grids.append(grid)

    return {
        "N": N, "NP": NP, "PR": PR, "SLOTS": SLOTS, "C": C,
        "chunks": chunks, "grids": grids, "slot_nodes": slot_nodes,
        "slot_of": slot_of, "deg": deg, "col_prow": col_prow,
    }


def _wrap_idx(q, chunks):
    """q: [128, C] int idx grid -> [128, C*8] int16 call-wrapped layout.

    Gather calls cover GPC grid columns starting at each chunk's c0; per
    call, flat k' = jl*128+p -> staged [k'%16, (c0+s0)*8 + k'//16],
    replicated across the 8 partition groups.
    """
    C = q.shape[1]
    out = np.zeros((16, C * 8), dtype=np.int16)
    for ch in chunks:
        for s0 in range(0, ch["width"], GPC):
            cw = min(GPC, ch["width"] - s0)
            g0 = ch["c0"] + s0
            flat = q[:, g0:g0 + cw].T.reshape(-1)      # k' = jl*128 + p
            blk = flat.reshape(cw * 8, 16).T           # [16, cw*8]
            out[:, g0 * 8:g0 * 8 + cw * 8] = blk
    return np.tile(out, (8, 1))                        # [128, C*8]


def _ag_pieces(PR):
    """AllGather piece boundaries in prows (staggered, small final piece so
    the next layer's gathers aren't stuck behind one big late collective).

    Returns (piece_lo_slots, piece_size_slots, piece_base_gids) as arrays.
    """
    cuts = sorted(set([0, min(49, PR), min(73, PR), min(87, PR),
                   min(95, PR), PR]))
    lo = np.array([c * 128 for c in cuts[:-1]], dtype=np.int64)
    sz = np.array([(cuts[i + 1] - cuts[i]) * 128 for i in range(len(cuts) - 1)],
                  dtype=np.int64)
    base = np.concatenate([[0], np.cumsum(sz * P)[:-1]])
    return lo, sz, base


def _build_core_inputs(meta, x):
    N, NP, PR, SLOTS, C = (meta[k] for k in ("N", "NP", "PR", "SLOTS", "C"))
    T2 = P * SLOTS
    inv = 1.0 / np.maximum(meta["deg"], 1).astype(np.float32)
    p_lo, p_sz, p_base = _ag_pieces(PR)

    # layer-1 feature table in AllGather gid order (same layout the kernel's
    # AllGather produces for layers 2/3, so one idx/mask set serves all)
    assert T2 % 4 == 0
    htbl = np.zeros((T2, F), np.float16)
    for c in range(P):
        sn = meta["slot_nodes"][c]
        for k in range(len(p_lo)):
            lo, sz = int(p_lo[k]), int(p_sz[k])
            slots = np.arange(lo, lo + sz)
            gids = int(p_base[k]) + c * sz + (slots - lo)
            real = sn[slots] >= 0
            htbl[gids[real]] = x[sn[slots][real]].astype(np.float16)

    per_core = []
    for c in range(P):
        grid = meta["grids"][c]                             # [128, C] node or -1
        valid = grid >= 0
        node = np.where(valid, grid, 0)

        owner = node // NP
        slot = meta["slot_of"][node]
        k = np.searchsorted(p_lo, slot, side="right") - 1
        gid = p_base[k] + owner * p_sz[k] + (slot - p_lo[k])
        q2 = (gid // 4).astype(np.int16)
        m2 = (gid % 4).astype(np.int64)

        # inv-degree of each cell's dst (by its (p, prow) position)
        slot_node = meta["slot_nodes"][c]
        prow = meta["col_prow"]                              # [C] prow of col
        dslot = prow[None, :] * 128 + np.arange(128)[:, None]  # [128, C] slot
        dn = slot_node[dslot]
        cinv = np.where(dn >= 0, inv[np.maximum(dn, 0)], 0.0).astype(np.float32)
        cinv = cinv * valid

        msk = np.zeros((128, C, 4), np.float16)
        pp, cc2 = np.nonzero(valid)
        msk[pp, cc2, m2[pp, cc2]] = cinv[pp, cc2]
        # duplicated feature-pair layout [128, C*4, 2] for 2x-mode DVE mul
        mskd = np.repeat(msk.reshape(128, C * 4, 1), 2, axis=2)

        rr = np.arange(SLOTS)
        real = slot_node >= 0
        xfm = np.zeros((F, SLOTS), np.float32)
        xfm[:, rr[real]] = x[slot_node[real]].T
        per_core.append({
            "idx": _wrap_idx(q2, meta["chunks"]),
            "msk": np.ascontiguousarray(mskd),
            "xfm": xfm, "htbl": htbl,
        })
    return per_core


# ---------------------------------------------------------------- bass builder

def _dma_gather_raw(gp, out_ap, in_ap, idxs_ap, num_idxs, elem_size,
                    elem_step=None, queue_num=0, num_idxs_reg=None):
    """bass dma_gather minus the elem%256B transpose-only restriction."""
    import concourse.mybir as mybir
    from concourse import ap_utils
    from concourse._compat import exact_div

    if num_idxs_reg is None:
        num_idxs_reg = num_idxs

    assert idxs_ap.dtype == mybir.dt.int16
    assert in_ap.dtype == out_ap.dtype
    if elem_step is None:
        assert ap_utils.ap_is_contiguous(in_ap.ap[1:])
        elem_step = elem_size
    assert ap_utils.ap_is_contiguous(out_ap.ap[1:])
    assert ap_utils.ap_is_contiguous(idxs_ap.ap[1:])
    assert in_ap.ap[-1][1] == out_ap.ap[-1][1] == elem_size
    assert out_ap.ap[0][1] * out_ap.ap[1][1] == num_idxs
    assert in_ap.ap[0][0] == elem_step
    stride_bytes_256 = exact_div(elem_step * mybir.dt.size(in_ap.dtype), 256)
    assert stride_bytes_256 < 256

    _in_ap = gp.lower_ap_dma(in_ap, for_custom_bir_dma=True)
    _idxs_ap = gp.lower_ap(idxs_ap)
    _out_ap = gp.lower_ap(out_ap)
    return gp.add_instruction(
        mybir.InstDMAGatherAnt(
            name=gp.bass.get_next_instruction_name(),
            ins=[*_in_ap, _idxs_ap,
                 gp.lower_val_access(gp.to_reg(num_idxs_reg))],
            outs=[_out_ap],
            transpose=False,
            num_idxs=num_idxs,
            elem_size=elem_size,
            stride_bytes_256=stride_bytes_256,
            gen_mode=0,
            single_packet=True,
            queue_num=queue_num,
            sbuf_tokens_per_rank=0,
            sbuf_free_dim_per_rank=0,
            sbuf_free_dim_pad_per_rank=0,
            sbuf_byte_offset=0,
        )
    )


def _build_bass(meta, n_cores=P):
    from concourse import bacc, tile, mybir

    N, NP, PR, SLOTS, C = (meta[k] for k in ("N", "NP", "PR", "SLOTS", "C"))
    T2 = P * SLOTS
    f32 = mybir.dt.float32
    f16 = mybir.dt.float16
    i16 = mybir.dt.int16
    AF = mybir.ActivationFunctionType
    OP = mybir.AluOpType
    AX = mybir.AxisListType

    nc = bacc.Bacc("TRN2", target_bir_lowering=False, debug=False,
                   num_devices=n_cores, num_swdge_queues=4,
                   dynamic_dma_scratch_size=8192)
    htbl = nc.dram_tensor("htbl", [T2, F], f16, kind="ExternalInput")
    idx_d = nc.dram_tensor("idx", [128, C * 8], i16, kind="ExternalInput")
    msk_d = nc.dram_tensor("msk", [128, C * 4, 2], f16, kind="ExternalInput")
    xfm_d = nc.dram_tensor("xfm", [F, SLOTS], f32, kind="ExternalInput")
    ident_d = nc.dram_tensor("ident", [128, 128], f32, kind="ExternalInput")
    ident16_d = nc.dram_tensor("ident16", [128, 128], f16, kind="ExternalInput")
    wts = {}
    for i in (1, 2, 3):
        wts[f"Wl{i}"] = nc.dram_tensor(f"Wl{i}", [F, F], f32, kind="ExternalInput")
        wts[f"Wr{i}"] = nc.dram_tensor(f"Wr{i}", [F, F], f32, kind="ExternalInput")
        wts[f"bl{i}"] = nc.dram_tensor(f"bl{i}", [F, 1], f32, kind="ExternalInput")
    wts["Wc"] = nc.dram_tensor("Wc", [F, NCLS], f32, kind="ExternalInput")
    wts["bc"] = nc.dram_tensor("bc", [128, NCLS], f32, kind="ExternalInput")
    out_d = nc.dram_tensor("out", [SLOTS, NCLS], f32, kind="ExternalOutput")

    maxpr = max(ch["rects"][-1]["i0"] + ch["rects"][-1]["m"]
                - ch["rects"][0]["i0"] for ch in meta["chunks"])
    pc_lo, pc_sz, pc_base = _ag_pieces(PR)

    with tile.TileContext(nc) as tc:
        from contextlib import ExitStack
        with ExitStack() as es:
            dram = es.enter_context(tc.tile_pool(name="dram", bufs=1, space="DRAM"))
            const = es.enter_context(tc.tile_pool(name="const", bufs=1))
            gbuf = es.enter_context(tc.tile_pool(name="gbuf", bufs=6))
            mpool = es.enter_context(tc.tile_pool(name="mpool", bufs=3))
            psT = es.enter_context(tc.tile_pool(name="psT", bufs=2, space="PSUM"))
            psM = es.enter_context(tc.tile_pool(name="psM", bufs=2, space="PSUM"))
            psN = es.enter_context(tc.tile_pool(name="psN", bufs=2, space="PSUM"))

            agin_t = dram.tile([SLOTS, F], f16, tag="agin", name="agin")
            agout_a = dram.tile([T2, F], f16, tag="agout_a", name="agout_a")
            agout_b = dram.tile([T2, F], f16, tag="agout_b", name="agout_b")
            hfm_a = dram.tile([F, SLOTS], f32, tag="hfm_a", name="hfm_a")
            hfm_b = dram.tile([F, SLOTS], f32, tag="hfm_b", name="hfm_b")
            hfm_dram = [hfm_a, hfm_b]

            idx_t = const.tile([128, C * 8], i16, tag="idx", name="idx_t")
            nc.sync.dma_start(idx_t[:], idx_d[:])
            msk_t = const.tile([128, C * 4, 2], f16, tag="msk", name="msk_t")
            nc.sync.dma_start(msk_t[:], msk_d[:])
            ident_t = const.tile([128, 128], f32, tag="ident", name="ident_t")
            nc.sync.dma_start(ident_t[:], ident_d[:])
            ident16_t = const.tile([128, 128], f16, tag="ident16",
                                   name="ident16_t")
            nc.sync.dma_start(ident16_t[:], ident16_d[:])
            w_t = {}
            for k, dten in wts.items():
                wtile = const.tile(list(dten.shape), f32, tag=k, name=f"w_{k}")
                w_t[k] = wtile
                nc.sync.dma_start(wtile[:], dten[:])

            mean_t = const.tile([128, PR, F], f16, tag="mean", name="mean_t")
            ngrid_t = const.tile([128, PR, F], f16, tag="ngrid", name="ngrid_t")
            ogrid_t = const.tile([128, PR, NCLS], f32, tag="ogrid",
                                 name="ogrid_t")
            ogrid2_t = const.tile([128, PR, NCLS], f32, tag="ogrid2",
                                  name="ogrid2_t")

            for L in range(3):
                if L == 0:
                    table_q = htbl[:].rearrange("(q g) f -> q (g f)", g=4)
                elif L == 1:
                    table_q = agout_a[:].rearrange("(q g) f -> q (g f)", g=4)
                else:
                    table_q = agout_b[:].rearrange("(q g) f -> q (g f)", g=4)
                agout_t = agout_a if L == 0 else agout_b
                hin = xfm_d if L == 0 else hfm_dram[(L + 1) % 2]
                hout = hfm_dram[L % 2]
                Wl, Wr, bl = w_t[f"Wl{L+1}"], w_t[f"Wr{L+1}"], w_t[f"bl{L+1}"]

                def send_piece(k):
                    lo = int(pc_lo[k])
                    szs = int(pc_sz[k])
                    base = int(pc_base[k])
                    pr_lo, pr_n = lo // 128, szs // 128
                    nc.sync.dma_start(
                        agin_t[lo:lo + szs, :].rearrange(
                            "(i p) f -> p i f", p=128),
                        ngrid_t[:, pr_lo:pr_lo + pr_n, :])
                    nc.gpsimd.collective_compute(
                        "AllGather", OP.bypass,
                        ins=[agin_t[lo:lo + szs, :].opt()],
                        outs=[agout_t[base:base + n_cores * szs, :].opt()],
                        replica_groups=[list(range(n_cores))])

                pieces_sent = 0
                qn = 0
                for ch in meta["chunks"]:
                    W = ch["width"]
                    c0 = ch["c0"]
                    buf = gbuf.tile([128, CW, QF], f16, tag="chunk", name="buf")
                    for s0 in range(0, W, GPC):
                        cw = min(GPC, W - s0)
                        _dma_gather_raw(
                            nc.gpsimd, buf[:, s0:s0 + cw, :], table_q,
                            idx_t[:, (c0 + s0) * 8:(c0 + s0 + cw) * 8],
                            cw * 128, QF, queue_num=qn)
                        qn = (qn + 1) % 4
                    # mask * inv-degree per chunk (zeroes junk quad rows +
                    # pad)
                    if MASK4D:
                        # duplicated-pair mask keeps every operand's innermost
                        # dim a packed 2-elem fp16 run -> DVE 2x mode
                        v = buf[:, 0:W, :].rearrange(
                            "p w (g f2 two) -> p (w g) f2 two", g=4, two=2)
                        mv = msk_t[:, c0 * 4:(c0 + W) * 4, :].rearrange(
                            "p m (one two) -> p m one two", one=1).broadcast_to(
                            [128, W * 4, F // 2, 2])
                        nc.vector.tensor_mul(v, v, mv)
                    else:
                        v = buf[:, 0:W, :].rearrange(
                            "p w (g f) -> p (w g) f", g=4)
                        nc.vector.tensor_mul(
                            v, v, msk_t[:, c0 * 4:(c0 + W) * 4, 0:1]
                            .broadcast_to([128, W * 4, F]))
                    # segment-reduce each rectangle by pairwise folding
                    for r in ch["rects"]:
                        m, w, i0 = r["m"], r["w"], r["i0"]
                        D = buf[:, r["coff"]:r["coff"] + m * w, :].rearrange(
                            "p (m w) (g f) -> p m (w g) f", m=m, w=w, g=4)
                        X = 4 * w
                        while X > 2:
                            h = (X + 1) // 2
                            lo = X - h
                            nc.vector.tensor_add(D[:, :, 0:lo, :],
                                                 D[:, :, 0:lo, :],
                                                 D[:, :, h:X, :])
                            X = h
                        nc.vector.tensor_add(mean_t[:, i0:i0 + m, :],
                                             D[:, :, 0, :], D[:, :, 1, :])
                    # this chunk's prows are final: transpose, matmul now
                    i0c = ch["rects"][0]["i0"]
                    i1c = ch["rects"][-1]["i0"] + ch["rects"][-1]["m"]
                    npr = i1c - i0c
                    s0 = i0c * 128
                    wd = npr * 128
                    mfm = mpool.tile([F, maxpr * 128], f32, tag="mfm", name="mfm")
                    for k in range(npr):
                        i = i0c + k
                        ps = psT.tile([F, 128], f16, tag="psT", name="psTt")
                        nc.tensor.transpose(ps[:], mean_t[:, i, :],
                                            ident16_t[:])
                        nc.scalar.activation(mfm[:, k * 128:(k + 1) * 128],
                                             ps[:], AF.Copy)
                    hin_sb = mpool.tile([F, maxpr * 128], f32, tag="hin",
                                        name="hin_sb")
                    nc.sync.dma_start(hin_sb[:, :wd], hin[:, s0:s0 + wd])
                    for q0 in range(0, wd, MMW):
                        qw = min(MMW, wd - q0)
                        ps = psM.tile([F, MMW], f32, tag="psM", name="psMt")
                        nc.tensor.matmul(ps[:, :qw], Wl[:],
                                         mfm[:, q0:q0 + qw],
                                         start=True, stop=False)
                        nc.tensor.matmul(ps[:, :qw], Wr[:],
                                         hin_sb[:, q0:q0 + qw],
                                         start=False, stop=True)
                        nc.scalar.activation(mfm[:, q0:q0 + qw], ps[:, :qw],
                                             AF.Relu, bias=bl[:])
                    hout_sb = mfm   # relu result written back into mfm tile
                    if L == 2:
                        for k in range(npr):
                            i = i0c + k
                            psc = psN.tile([128, NCLS], f32, tag="psN",
                                           name="psct")
                            nc.tensor.matmul(
                                psc[:], hout_sb[:, k * 128:(k + 1) * 128],
                                w_t["Wc"][:], start=True, stop=True)
                            nc.vector.tensor_add(ogrid_t[:, i, :], psc[:],
                                                 w_t["bc"][:])
                    else:
                        if s0 + wd > NP:
                            zoff = max(0, NP - s0)
                            nc.vector.memset(hout_sb[:, zoff:wd], 0.0)
                        nc.sync.dma_start(hout[:, s0:s0 + wd], hout_sb[:, :wd])
                        for k in range(npr):
                            i = i0c + k
                            psn = psN.tile([128, F], f32, tag="psN", name="psnt")
                            nc.tensor.transpose(
                                psn[:], hout_sb[:, k * 128:(k + 1) * 128],
                                ident_t[:F, :F])
                            nc.scalar.activation(ngrid_t[:, i, :], psn[:],
                                                 AF.Copy)
                        while (pieces_sent < len(pc_lo) - 1 and
                               i1c * 128 >= pc_lo[pieces_sent] + pc_sz[pieces_sent]):
                            send_piece(pieces_sent)
                            pieces_sent += 1

                if L < 2:
                    while pieces_sent < len(pc_lo):
                        send_piece(pieces_sent)
                        pieces_sent += 1

            mx = const.tile([128, PR, 1], f32, tag="mx", name="mx")
            nc.vector.tensor_reduce(mx[:], ogrid_t[:], AX.X, OP.max)
            nc.vector.tensor_sub(ogrid2_t[:], ogrid_t[:],
                                 mx[:].broadcast_to([128, PR, NCLS]))
            eg = const.tile([128, PR, NCLS], f32, tag="eg", name="eg")
            nc.scalar.activation(eg[:], ogrid2_t[:], AF.Exp)
            sm = const.tile([128, PR, 1], f32, tag="sm", name="sm")
            nc.vector.tensor_reduce(sm[:], eg[:], AX.X, OP.add)
            lsm = const.tile([128, PR, 1], f32, tag="lsm", name="lsm")
            nc.scalar.activation(lsm[:], sm[:], AF.Ln)
            nc.vector.tensor_sub(ogrid_t[:], ogrid2_t[:],
                                 lsm[:].broadcast_to([128, PR, NCLS]))
            nc.sync.dma_start(out_d[:].rearrange("(i p) c -> p i c", p=128),
                              ogrid_t[:])
    nc.compile()
    return nc


def _install_ntff_hook():
    mod = types.ModuleType("antenv.axon_hooks")
    def s(h):
        mod._hook = h
    def g():
        return getattr(mod, "_hook", None)
    mod.set_axon_ntff_profile_hook = s
    mod.get_axon_ntff_profile_hook = g
    sys.modules["antenv.axon_hooks"] = mod
    import antenv
    antenv.axon_hooks = mod
    from trn_agent_boot.trn_boot import _ntff_profile_via_ctypes
    s(_ntff_profile_via_ctypes("/opt/axon/libaxon_pjrt.so"))


def kernel(**inputs):
    global LAST_EXEC_NS
    from concourse import bass_utils
    from concourse.bass_interp import get_hw_module

    x = np.asarray(inputs["x"], np.float32)
    edge_index = np.asarray(inputs["edge_index"], np.int64)
    N = x.shape[0]

    meta = _build_meta(edge_index, N)
    per_core = _build_core_inputs(meta, x)
    nc = _build_bass(meta, n_cores=P)
    nc.m = get_hw_module(nc.m)

    ident = np.eye(128, dtype=np.float32)
    ins = []
    for c in range(P):
        pc = per_core[c]
        m = {"htbl": pc["htbl"], "idx": pc["idx"], "msk": pc["msk"],
             "xfm": pc["xfm"], "ident": ident,
             "ident16": ident.astype(np.float16)}
        for i in (1, 2, 3):
            m[f"Wl{i}"] = np.asarray(inputs[f"Wl{i}"], np.float32)
            m[f"Wr{i}"] = np.asarray(inputs[f"Wr{i}"], np.float32)
            m[f"bl{i}"] = np.asarray(inputs[f"bl{i}"],
                                     np.float32).reshape(F, 1)
        m["Wc"] = np.asarray(inputs["Wc"], np.float32)
        m["bc"] = np.tile(np.asarray(inputs["bc"], np.float32).reshape(1, NCLS),
                          (128, 1))
        ins.append(m)

    trace = os.environ.get("KERNEL_TRACE", "0") == "1"
    if trace:
        try:
            _install_ntff_hook()
        except Exception:
            trace = False
    res = bass_utils.run_bass_kernel_spmd(
        nc, ins, core_ids=list(range(P)), trace=trace)
    LAST_EXEC_NS = res.exec_time_ns

    full = np.zeros((N, NCLS), np.float32)
    for c in range(P):
        sn = meta["slot_nodes"][c]
        real = sn >= 0
        full[sn[real]] = res.results[c]["out"][real]
    return full



# revision 44
# speedup vs baseline: 1.3738x; 1.0147x over previous
"""GraphSAGE (3x SAGEConv mean-agg + linear classifier + log_softmax) on 8
Trainium2 NeuronCores via Bass.

Self-contained: host-side packing + SPMD bass program + gather/unshard.

Sharding: nodes are dst-sharded 8 ways (core c owns nodes [c*NP, (c+1)*NP)).
Per layer, each core:
  - bulk-gathers its in-edges' source rows from a replicated fp16 DRAM
    feature table with InstDMAGatherAnt (1024 int16 indices per call, quad
    granularity: each 512B descriptor fetches 4 rows, the wanted one selected
    by a per-cell mask),
  - applies mask*(1/deg) on the Vector engine (broadcast multiply), then
    segment-reduces each degree-sorted rectangle with a contiguous
    pairwise fold tree,
  - PE-transposes the mean grid to feature-major, matmuls Wl/Wr with PSUM
    accumulation, applies bias+ReLU on the Scalar engine,
  - PE-transposes back to node-major (fp16) and AllGathers the shard into
    the next layer's fp16 table (compute/slot order).
The tiny 64-wide linears are replicated on every core.
"""
import os
import sys
import types

import numpy as np

sys.path.insert(0, "/opt/trn_rl_repo")

P = 8
F = 64
NCLS = 10
CW = 36          # gather-chunk width (grid columns per SBUF chunk buffer)
GPC = 8          # grid columns per dma_gather call (8*128 = 1024 idx)
MMW = 512        # matmul moving-chunk width
QF = 256         # fp16 elems per quad cell (4 rows x 64)
MASK4D = int(os.environ.get("MASK4D", "1")) == 1

LAST_EXEC_NS = None


# ---------------------------------------------------------------- host packing

def _build_meta(edge_index, n_nodes):
    N = n_nodes
    NP = N // P
    PR = (NP + 127) // 128
    SLOTS = PR * 128
    assert NP < SLOTS, "need at least one dummy slot (NP not divisible by 128)"
    src = np.asarray(edge_index[0], dtype=np.int64)
    dst = np.asarray(edge_index[1], dtype=np.int64)
    deg = np.bincount(dst, minlength=N).astype(np.int64)

    orders = []
    deg_sorted = []
    for c in range(P):
        dl = deg[c * NP:(c + 1) * NP]
        o = np.argsort(-dl, kind="stable")
        orders.append(o)
        deg_sorted.append(dl[o])
    w_prow = np.zeros(PR, dtype=np.int64)
    for i in range(PR):
        w_prow[i] = max(1, max(int(ds[i * 128]) for ds in deg_sorted))

    # pack prow-rects into chunks of width exactly CW (multiple of GPC) so
    # every dma_gather call is full-width; equal-w prow runs are split freely
    chunks = []
    col = 0
    cur = {"c0": 0, "width": 0, "rects": []}
    i = 0
    while i < PR:
        w = int(w_prow[i])
        assert w <= CW, f"prow width {w} exceeds chunk width {CW}"
        if cur["width"] + w > CW:
            chunks.append(cur)
            cur = {"c0": col, "width": 0, "rects": []}
        m = 1
        while (i + m < PR and int(w_prow[i + m]) == w
               and cur["width"] + (m + 1) * w <= CW):
            m += 1
        cur["rects"].append({"coff": cur["width"], "i0": i, "m": m, "w": w})
        cur["width"] += m * w
        col += m * w
        i += m
    if cur["width"] > 0:
        chunks.append(cur)
    C = col

    cc = 0
    for ch in chunks:
        ch["c0"] = cc
        cc += ch["width"]
    c0_prow = np.zeros(PR, dtype=np.int64)
    col_prow = np.zeros(C, dtype=np.int64)
    for ch in chunks:
        for r in ch["rects"]:
            for k in range(r["m"]):
                c0 = ch["c0"] + r["coff"] + k * r["w"]
                c0_prow[r["i0"] + k] = c0
                col_prow[c0:c0 + r["w"]] = r["i0"] + k

    grids = []
    slot_nodes = []
    slot_of = np.full(N, -1, dtype=np.int64)
    for c in range(P):
        o = orders[c]
        slot_node = np.full(SLOTS, -1, dtype=np.int64)
        slot_node[:NP] = o + c * NP
        slot_nodes.append(slot_node)
        slot_of[o + c * NP] = np.arange(NP)

        grid = np.full((128, C), -1, dtype=np.int64)
        m = (dst >= c * NP) & (dst < (c + 1) * NP)
        es, ed = src[m], dst[m] - c * NP
        eo = np.argsort(ed, kind="stable")
        es, ed = es[eo], ed[eo]
        estart = np.zeros(NP + 1, dtype=np.int64)
        np.cumsum(np.bincount(ed, minlength=NP), out=estart[1:])
        r_e = slot_of[ed + c * NP]
        k_e = np.arange(es.shape[0]) - estart[ed]
        pp = r_e % 128
        cols = c0_prow[r_e // 128] + k_e
        grid[pp, cols] = es
        grids.append(grid)

    return {
        "N": N, "NP": NP, "PR": PR, "SLOTS": SLOTS, "C": C,
        "chunks": chunks, "grids": grids, "slot_nodes": slot_nodes,
        "slot_of": slot_of, "deg": deg, "col_prow": col_prow,
    }


def _wrap_idx(q, chunks):
    """q: [128, C] int idx grid -> [128, C*8] int16 call-wrapped layout.

    Gather calls cover GPC grid columns starting at each chunk's c0; per
    call, flat k' = jl*128+p -> staged [k'%16, (c0+s0)*8 + k'//16],
    replicated across the 8 partition groups.
    """
    C = q.shape[1]
    out = np.zeros((16, C * 8), dtype=np.int16)
    for ch in chunks:
        for s0 in range(0, ch["width"], GPC):
            cw = min(GPC, ch["width"] - s0)
            g0 = ch["c0"] + s0
            flat = q[:, g0:g0 + cw].T.reshape(-1)      # k' = jl*128 + p
            blk = flat.reshape(cw * 8, 16).T           # [16, cw*8]
            out[:, g0 * 8:g0 * 8 + cw * 8] = blk
    return np.tile(out, (8, 1))                        # [128, C*8]


def _ag_pieces(PR):
    """AllGather piece boundaries in prows (staggered, small final piece so
    the next layer's gathers aren't stuck behind one big late collective).

    Returns (piece_lo_slots, piece_size_slots, piece_base_gids) as arrays.
    """
    cuts = sorted(set(min(c, PR) for c in
                      [0, 20, 38, 52, 63, 73, 82, 90, 95, PR]))
    lo = np.array([c * 128 for c in cuts[:-1]], dtype=np.int64)
    sz = np.array([(cuts[i + 1] - cuts[i]) * 128 for i in range(len(cuts) - 1)],
                  dtype=np.int64)
    base = np.concatenate([[0], np.cumsum(sz * P)[:-1]])
    return lo, sz, base


def _build_core_inputs(meta, x):
    N, NP, PR, SLOTS, C = (meta[k] for k in ("N", "NP", "PR", "SLOTS", "C"))
    T2 = P * SLOTS
    inv = 1.0 / np.maximum(meta["deg"], 1).astype(np.float32)
    p_lo, p_sz, p_base = _ag_pieces(PR)

    # layer-1 feature table in AllGather gid order (same layout the kernel's
    # AllGather produces for layers 2/3, so one idx/mask set serves all)
    assert T2 % 4 == 0
    htbl = np.zeros((T2, F), np.float16)
    for c in range(P):
        sn = meta["slot_nodes"][c]
        for k in range(len(p_lo)):
            lo, sz = int(p_lo[k]), int(p_sz[k])
            slots = np.arange(lo, lo + sz)
            gids = int(p_base[k]) + c * sz + (slots - lo)
            real = sn[slots] >= 0
            htbl[gids[real]] = x[sn[slots][real]].astype(np.float16)

    per_core = []
    for c in range(P):
        grid = meta["grids"][c]                             # [128, C] node or -1
        valid = grid >= 0
        node = np.where(valid, grid, 0)

        owner = node // NP
        slot = meta["slot_of"][node]
        k = np.searchsorted(p_lo, slot, side="right") - 1
        gid = p_base[k] + owner * p_sz[k] + (slot - p_lo[k])
        q2 = (gid // 4).astype(np.int16)
        m2 = (gid % 4).astype(np.int64)

        # inv-degree of each cell's dst (by its (p, prow) position)
        slot_node = meta["slot_nodes"][c]
        prow = meta["col_prow"]                              # [C] prow of col
        dslot = prow[None, :] * 128 + np.arange(128)[:, None]  # [128, C] slot
        dn = slot_node[dslot]
        cinv = np.where(dn >= 0, inv[np.maximum(dn, 0)], 0.0).astype(np.float32)
        cinv = cinv * valid

        msk = np.zeros((128, C, 4), np.float16)
        pp, cc2 = np.nonzero(valid)
        msk[pp, cc2, m2[pp, cc2]] = cinv[pp, cc2]
        # duplicated feature-pair layout [128, C*4, 2] for 2x-mode DVE mul
        mskd = np.repeat(msk.reshape(128, C * 4, 1), 2, axis=2)

        rr = np.arange(SLOTS)
        real = slot_node >= 0
        xfm = np.zeros((F, SLOTS), np.float32)
        xfm[:, rr[real]] = x[slot_node[real]].T
        per_core.append({
            "idx": _wrap_idx(q2, meta["chunks"]),
            "msk": np.ascontiguousarray(mskd),
            "xfm": xfm, "htbl": htbl,
        })
    return per_core


# ---------------------------------------------------------------- bass builder

def _dma_gather_raw(gp, out_ap, in_ap, idxs_ap, num_idxs, elem_size,
                    elem_step=None, queue_num=0, num_idxs_reg=None):
    """bass dma_gather minus the elem%256B transpose-only restriction."""
    import concourse.mybir as mybir
    from concourse import ap_utils
    from concourse._compat import exact_div

    if num_idxs_reg is None:
        num_idxs_reg = num_idxs

    assert idxs_ap.dtype == mybir.dt.int16
    assert in_ap.dtype == out_ap.dtype
    if elem_step is None:
        assert ap_utils.ap_is_contiguous(in_ap.ap[1:])
        elem_step = elem_size
    assert ap_utils.ap_is_contiguous(out_ap.ap[1:])
    assert ap_utils.ap_is_contiguous(idxs_ap.ap[1:])
    assert in_ap.ap[-1][1] == out_ap.ap[-1][1] == elem_size
    assert out_ap.ap[0][1] * out_ap.ap[1][1] == num_idxs
    assert in_ap.ap[0][0] == elem_step
    stride_bytes_256 = exact_div(elem_step * mybir.dt.size(in_ap.dtype), 256)
    assert stride_bytes_256 < 256

    _in_ap = gp.lower_ap_dma(in_ap, for_custom_bir_dma=True)
    _idxs_ap = gp.lower_ap(idxs_ap)
    _out_ap = gp.lower_ap(out_ap)
    return gp.add_instruction(
        mybir.InstDMAGatherAnt(
            name=gp.bass.get_next_instruction_name(),
            ins=[*_in_ap, _idxs_ap,
                 gp.lower_val_access(gp.to_reg(num_idxs_reg))],
            outs=[_out_ap],
            transpose=False,
            num_idxs=num_idxs,
            elem_size=elem_size,
            stride_bytes_256=stride_bytes_256,
            gen_mode=0,
            single_packet=True,
            queue_num=queue_num,
            sbuf_tokens_per_rank=0,
            sbuf_free_dim_per_rank=0,
            sbuf_free_dim_pad_per_rank=0,
            sbuf_byte_offset=0,
        )
    )


def _build_bass(meta, n_cores=P):
    from concourse import bacc, tile, mybir

    N, NP, PR, SLOTS, C = (meta[k] for k in ("N", "NP", "PR", "SLOTS", "C"))
    T2 = P * SLOTS
    f32 = mybir.dt.float32
    f16 = mybir.dt.float16
    i16 = mybir.dt.int16
    AF = mybir.ActivationFunctionType
    OP = mybir.AluOpType
    AX = mybir.AxisListType

    nc = bacc.Bacc("TRN2", target_bir_lowering=False, debug=False,
                   num_devices=n_cores, num_swdge_queues=4,
                   dynamic_dma_scratch_size=4096)
    htbl = nc.dram_tensor("htbl", [T2, F], f16, kind="ExternalInput")
    idx_d = nc.dram_tensor("idx", [128, C * 8], i16, kind="ExternalInput")
    msk_d = nc.dram_tensor("msk", [128, C * 4, 2], f16, kind="ExternalInput")
    xfm_d = nc.dram_tensor("xfm", [F, SLOTS], f32, kind="ExternalInput")
    ident_d = nc.dram_tensor("ident", [128, 128], f32, kind="ExternalInput")
    ident16_d = nc.dram_tensor("ident16", [128, 128], f16, kind="ExternalInput")
    wts = {}
    for i in (1, 2, 3):
        wts[f"Wl{i}"] = nc.dram_tensor(f"Wl{i}", [F, F], f32, kind="ExternalInput")
        wts[f"Wr{i}"] = nc.dram_tensor(f"Wr{i}", [F, F], f32, kind="ExternalInput")
        wts[f"bl{i}"] = nc.dram_tensor(f"bl{i}", [F, 1], f32, kind="ExternalInput")
    wts["Wc"] = nc.dram_tensor("Wc", [F, NCLS], f32, kind="ExternalInput")
    wts["bc"] = nc.dram_tensor("bc", [128, NCLS], f32, kind="ExternalInput")
    out_d = nc.dram_tensor("out", [SLOTS, NCLS], f32, kind="ExternalOutput")

    maxpr = max(ch["rects"][-1]["i0"] + ch["rects"][-1]["m"]
                - ch["rects"][0]["i0"] for ch in meta["chunks"])
    pc_lo, pc_sz, pc_base = _ag_pieces(PR)

    with tile.TileContext(nc) as tc:
        from contextlib import ExitStack
        with ExitStack() as es:
            dram = es.enter_context(tc.tile_pool(name="dram", bufs=1, space="DRAM"))
            const = es.enter_context(tc.tile_pool(name="const", bufs=1))
            gbuf = es.enter_context(tc.tile_pool(name="gbuf", bufs=5))
            mpool = es.enter_context(tc.tile_pool(name="mpool", bufs=3))
            psT = es.enter_context(tc.tile_pool(name="psT", bufs=2, space="PSUM"))
            psM = es.enter_context(tc.tile_pool(name="psM", bufs=2, space="PSUM"))
            psN = es.enter_context(tc.tile_pool(name="psN", bufs=2, space="PSUM"))

            agin_t = dram.tile([SLOTS, F], f16, tag="agin", name="agin")
            agout_a = dram.tile([T2, F], f16, tag="agout_a", name="agout_a")
            agout_b = dram.tile([T2, F], f16, tag="agout_b", name="agout_b")
            hfm_a = dram.tile([F, SLOTS], f32, tag="hfm_a", name="hfm_a")
            hfm_b = dram.tile([F, SLOTS], f32, tag="hfm_b", name="hfm_b")
            hfm_dram = [hfm_a, hfm_b]

            idx_t = const.tile([128, C * 8], i16, tag="idx", name="idx_t")
            nc.sync.dma_start(idx_t[:], idx_d[:])
            msk_t = const.tile([128, C * 4, 2], f16, tag="msk", name="msk_t")
            nc.sync.dma_start(msk_t[:], msk_d[:])
            ident_t = const.tile([128, 128], f32, tag="ident", name="ident_t")
            nc.sync.dma_start(ident_t[:], ident_d[:])
            ident16_t = const.tile([128, 128], f16, tag="ident16",
                                   name="ident16_t")
            nc.sync.dma_start(ident16_t[:], ident16_d[:])
            w_t = {}
            for k, dten in wts.items():
                wtile = const.tile(list(dten.shape), f32, tag=k, name=f"w_{k}")
                w_t[k] = wtile
                nc.sync.dma_start(wtile[:], dten[:])

            mean_t = const.tile([128, PR, F], f16, tag="mean", name="mean_t")
            ngrid_t = const.tile([128, PR, F], f16, tag="ngrid", name="ngrid_t")
            ogrid_t = const.tile([128, PR, NCLS], f32, tag="ogrid",
                                 name="ogrid_t")
            ogrid2_t = const.tile([128, PR, NCLS], f32, tag="ogrid2",
                                  name="ogrid2_t")

            for L in range(3):
                if L == 0:
                    table_q = htbl[:].rearrange("(q g) f -> q (g f)", g=4)
                elif L == 1:
                    table_q = agout_a[:].rearrange("(q g) f -> q (g f)", g=4)
                else:
                    table_q = agout_b[:].rearrange("(q g) f -> q (g f)", g=4)
                agout_t = agout_a if L == 0 else agout_b
                hin = xfm_d if L == 0 else hfm_dram[(L + 1) % 2]
                hout = hfm_dram[L % 2]
                Wl, Wr, bl = w_t[f"Wl{L+1}"], w_t[f"Wr{L+1}"], w_t[f"bl{L+1}"]

                def send_piece(k):
                    lo = int(pc_lo[k])
                    szs = int(pc_sz[k])
                    base = int(pc_base[k])
                    pr_lo, pr_n = lo // 128, szs // 128
                    nc.sync.dma_start(
                        agin_t[lo:lo + szs, :].rearrange(
                            "(i p) f -> p i f", p=128),
                        ngrid_t[:, pr_lo:pr_lo + pr_n, :])
                    nc.gpsimd.collective_compute(
                        "AllGather", OP.bypass,
                        ins=[agin_t[lo:lo + szs, :].opt()],
                        outs=[agout_t[base:base + n_cores * szs, :].opt()],
                        replica_groups=[list(range(n_cores))])

                pieces_sent = 0
                qn = 0
                for ch in meta["chunks"]:
                    W = ch["width"]
                    c0 = ch["c0"]
                    buf = gbuf.tile([128, CW, QF], f16, tag="chunk", name="buf")
                    for s0 in range(0, W, GPC):
                        cw = min(GPC, W - s0)
                        _dma_gather_raw(
                            nc.gpsimd, buf[:, s0:s0 + cw, :], table_q,
                            idx_t[:, (c0 + s0) * 8:(c0 + s0 + cw) * 8],
                            cw * 128, QF, queue_num=qn)
                        qn = (qn + 1) % 4
                    # mask * inv-degree per chunk (zeroes junk quad rows +
                    # pad)
                    if MASK4D:
                        # duplicated-pair mask keeps every operand's innermost
                        # dim a packed 2-elem fp16 run -> DVE 2x mode
                        v = buf[:, 0:W, :].rearrange(
                            "p w (g f2 two) -> p (w g) f2 two", g=4, two=2)
                        mv = msk_t[:, c0 * 4:(c0 + W) * 4, :].rearrange(
                            "p m (one two) -> p m one two", one=1).broadcast_to(
                            [128, W * 4, F // 2, 2])
                        nc.vector.tensor_mul(v, v, mv)
                    else:
                        v = buf[:, 0:W, :].rearrange(
                            "p w (g f) -> p (w g) f", g=4)
                        nc.vector.tensor_mul(
                            v, v, msk_t[:, c0 * 4:(c0 + W) * 4, 0:1]
                            .broadcast_to([128, W * 4, F]))
                    # segment-reduce each rectangle by pairwise folding
                    for r in ch["rects"]:
                        m, w, i0 = r["m"], r["w"], r["i0"]
                        D = buf[:, r["coff"]:r["coff"] + m * w, :].rearrange(
                            "p (m w) (g f) -> p m (w g) f", m=m, w=w, g=4)
                        X = 4 * w
                        while X > 2:
                            h = (X + 1) // 2
                            lo = X - h
                            nc.vector.tensor_add(D[:, :, 0:lo, :],
                                                 D[:, :, 0:lo, :],
                                                 D[:, :, h:X, :])
                            X = h
                        nc.vector.tensor_add(mean_t[:, i0:i0 + m, :],
                                             D[:, :, 0, :], D[:, :, 1, :])
                    # this chunk's prows are final: transpose, matmul now
                    i0c = ch["rects"][0]["i0"]
                    i1c = ch["rects"][-1]["i0"] + ch["rects"][-1]["m"]
                    npr = i1c - i0c
                    s0 = i0c * 128
                    wd = npr * 128
                    mfm = mpool.tile([F, maxpr * 128], f32, tag="mfm", name="mfm")
                    for k in range(npr):
                        i = i0c + k
                        ps = psT.tile([F, 128], f16, tag="psT", name="psTt")
                        nc.tensor.transpose(ps[:], mean_t[:, i, :],
                                            ident16_t[:])
                        nc.scalar.activation(mfm[:, k * 128:(k + 1) * 128],
                                             ps[:], AF.Copy)
                    hin_sb = mpool.tile([F, maxpr * 128], f32, tag="hin",
                                        name="hin_sb")
                    nc.sync.dma_start(hin_sb[:, :wd], hin[:, s0:s0 + wd])
                    for q0 in range(0, wd, MMW):
                        qw = min(MMW, wd - q0)
                        ps = psM.tile([F, MMW], f32, tag="psM", name="psMt")
                        nc.tensor.matmul(ps[:, :qw], Wl[:],
                                         mfm[:, q0:q0 + qw],
                                         start=True, stop=False)
                        nc.tensor.matmul(ps[:, :qw], Wr[:],
                                         hin_sb[:, q0:q0 + qw],
                                         start=False, stop=True)
                        nc.scalar.activation(mfm[:, q0:q0 + qw], ps[:, :qw],
                                             AF.Relu, bias=bl[:])
                    hout_sb = mfm   # relu result written back into mfm tile
                    if L == 2:
                        for k in range(npr):
                            i = i0c + k
                            psc = psN.tile([128, NCLS], f32, tag="psN",
                                           name="psct")
                            nc.tensor.matmul(
                                psc[:], hout_sb[:, k * 128:(k + 1) * 128],
                                w_t["Wc"][:], start=True, stop=True)
                            nc.vector.tensor_add(ogrid_t[:, i, :], psc[:],
                                                 w_t["bc"][:])
                    else:
                        if s0 + wd > NP:
                            zoff = max(0, NP - s0)
                            nc.vector.memset(hout_sb[:, zoff:wd], 0.0)
                        nc.sync.dma_start(hout[:, s0:s0 + wd], hout_sb[:, :wd])
                        for k in range(npr):
                            i = i0c + k
                            psn = psN.tile([128, F], f32, tag="psN", name="psnt")
                            nc.tensor.transpose(
                                psn[:], hout_sb[:, k * 128:(k + 1) * 128],
                                ident_t[:F, :F])
                            nc.scalar.activation(ngrid_t[:, i, :], psn[:],
                                                 AF.Copy)
                        while (pieces_sent < len(pc_lo) - 1 and
                               i1c * 128 >= pc_lo[pieces_sent] + pc_sz[pieces_sent]):
                            send_piece(pieces_sent)
                            pieces_sent += 1

                if L < 2:
                    while pieces_sent < len(pc_lo):
                        send_piece(pieces_sent)
                        pieces_sent += 1

            mx = const.tile([128, PR, 1], f32, tag="mx", name="mx")
            nc.vector.tensor_reduce(mx[:], ogrid_t[:], AX.X, OP.max)
            nc.vector.tensor_sub(ogrid2_t[:], ogrid_t[:],
                                 mx[:].broadcast_to([128, PR, NCLS]))
            eg = const.tile([128, PR, NCLS], f32, tag="eg", name="eg")
            nc.scalar.activation(eg[:], ogrid2_t[:], AF.Exp)
            sm = const.tile([128, PR, 1], f32, tag="sm", name="sm")
            nc.vector.tensor_reduce(sm[:], eg[:], AX.X, OP.add)
            lsm = const.tile([128, PR, 1], f32, tag="lsm", name="lsm")
            nc.scalar.activation(lsm[:], sm[:], AF.Ln)
            nc.vector.tensor_sub(ogrid_t[:], ogrid2_t[:],
                                 lsm[:].broadcast_to([128, PR, NCLS]))
            nc.sync.dma_start(out_d[:].rearrange("(i p) c -> p i c", p=128),
                              ogrid_t[:])
    nc.compile()
    return nc


def _install_ntff_hook():
    mod = types.ModuleType("antenv.axon_hooks")
    def s(h):
        mod._hook = h
    def g():
        return getattr(mod, "_hook", None)
    mod.set_axon_ntff_profile_hook = s
    mod.get_axon_ntff_profile_hook = g
    sys.modules["antenv.axon_hooks"] = mod
    import antenv
    antenv.axon_hooks = mod
    from trn_agent_boot.trn_boot import _ntff_profile_via_ctypes
    s(_ntff_profile_via_ctypes("/opt/axon/libaxon_pjrt.so"))


def kernel(**inputs):
    global LAST_EXEC_NS
    from concourse import bass_utils
    from concourse.bass_interp import get_hw_module

    x = np.asarray(inputs["x"], np.float32)
    edge_index = np.asarray(inputs["edge_index"], np.int64)
    N = x.shape[0]

    meta = _build_meta(edge_index, N)
    per_core = _build_core_inputs(meta, x)
    nc = _build_bass(meta, n_cores=P)
    nc.m = get_hw_module(nc.m)

    ident = np.eye(128, dtype=np.float32)
    ins = []
    for c in range(P):
        pc = per_core[c]
        m = {"htbl": pc["htbl"], "idx": pc["idx"], "msk": pc["msk"],
             "xfm": pc["xfm"], "ident": ident,
             "ident16": ident.astype(np.float16)}
        for i in (1, 2, 3):
            m[f"Wl{i}"] = np.asarray(inputs[f"Wl{i}"], np.float32)
            m[f"Wr{i}"] = np.asarray(inputs[f"Wr{i}"], np.float32)
            m[f"bl{i}"] = np.asarray(inputs[f"bl{i}"],
                                     np.float32).reshape(F, 1)
        m["Wc"] = np.asarray(inputs["Wc"], np.float32)
        m["bc"] = np.tile(np.asarray(inputs["bc"], np.float32).reshape(1, NCLS),
                          (128, 1))
        ins.append(m)

    trace = os.environ.get("KERNEL_TRACE", "0") == "1"
    if trace:
        try:
            _install_ntff_hook()
        except Exception:
            trace = False
    res = bass_utils.run_bass_kernel_spmd(
        nc, ins, core_ids=list(range(P)), trace=trace)
    LAST_EXEC_NS = res.exec_time_ns

    full = np.zeros((N, NCLS), np.float32)
    for c in range(P):
        sn = meta["slot_nodes"][c]
        real = sn >= 0
        full[sn[real]] = res.results[c]["out"][real]
    return full

